# revision 29
# baseline (speedup 1.0000x reference)
"""GAT (2-layer, 3-head) forward on 8 Trainium2 NeuronCores.

Math: with LeakyReLU slope ALPHA=1.0 the edge score e_ij = s1_i + s2_j is
linear, and s1_i cancels inside the row softmax.  The masked softmax over
j therefore reduces to column weights w_j = exp(s2_j - C) restricted to
adj, giving

    h'_i = (sum_j adj_ij * w_j * h_j) / (sum_j adj_ij * w_j)

i.e. one adjacency matmul against G = w*h plus a thin denominator matmul
against the w columns.  Both GAT layers take this form.

Precision: tolerance is 2e-2 so the adjacency matmuls run in fp8 e4m3
DoubleRow mode (2x effective PE rate: 256-deep contraction per pass) with
G scaled by 16 to lift small w*h products out of the subnormal range;
projections (x@W, xcat@Wo) run in single bf16.  End-to-end rel err ~3e-3.

Sharding: rows of h' (nodes) across 8 cores; each core holds fp8
lhsT-layout adjacency columns A^T[:, slab] and computes its 512-row slab.
Per-head fp8 G slabs are AllGathered so gathers pipeline against the
adjacency matmuls; head 0's gather is split in two halves so its L1
matmul starts sooner.  Layer 2 avoids a serial softmax-max collective by
building g2 with the core-local max and rescaling the gathered slabs by
exp(C_local - C_global) (the 8 local maxes travel in a tiny parallel
gather).

Schedule: x loads and the tiny s2 store/gather go first on uncontended
queues (vector issues the small critical-path DMAs) so the s2 AllGather
completes while the bulk W/adj/Wo loads stream in the background.
"""
import sys

sys.path.insert(0, "/opt/trn_rl_repo")

import math
import numpy as np
import ml_dtypes

import concourse.bass as bass
import concourse.bacc as bacc
import concourse.mybir as mybir
import concourse.bass_isa as bass_isa
import concourse.tile as tile
from concourse.bass_utils import run_bass_kernel_spmd

BF16 = ml_dtypes.bfloat16
F8 = ml_dtypes.float8_e4m3fn

N = 4096
F = 768
HID = 768
NH = 3
NCLS = 256
NCORES = 8
SLAB = N // NCORES          # 512 rows per core
NIT = SLAB // 128           # 4 i-tiles per core
NJT = N // 128              # 32 j-tiles
NQT = NJT // 2              # 16 j-tile pairs (DoubleRow)
NFT = F // 128              # 6 f-tiles
NCT = HID // 128            # 6 feature col-tiles of G per head
GH = NH * HID               # 2304 xcat columns
NOT = GH // 128             # 18 xcat col-tiles
G2C = NCLS + 1              # 257 = classes + w2 column
PAD2 = 264                  # G2 row padded to 8B
LOGC = math.log(16.0)       # fp8 scale for G / w columns (cancels in num/den)

AF = mybir.ActivationFunctionType
ALU = mybir.AluOpType
DR = mybir.MatmulPerfMode.DoubleRow


def build():
    dt = mybir.dt
    nc = bacc.Bacc(num_devices=NCORES)

    adjT_d = nc.dram_tensor("adjT", [N, SLAB], dt.float8e4, kind="ExternalInput")
    xT_d = nc.dram_tensor("xT", [F, SLAB], dt.bfloat16, kind="ExternalInput")
    U_d = nc.dram_tensor("U", [F, 8], dt.bfloat16, kind="ExternalInput")
    W_d = nc.dram_tensor("W", [NH, F, HID], dt.bfloat16, kind="ExternalInput")
    Wo_d = nc.dram_tensor("Wo", [GH, G2C], dt.float8e4, kind="ExternalInput")
    EYE_d = nc.dram_tensor("EYE", [128, 128], dt.float32, kind="ExternalInput")
    out_d = nc.dram_tensor("out", [SLAB, NCLS], dt.float32, kind="ExternalOutput")

    # DRAM scratch + collective buffers
    s2s_d = nc.dram_tensor("s2s", [SLAB, 4], dt.float32)
    s2f_d = nc.dram_tensor("s2f", [N, 4], dt.float32, addr_space="Shared")
    # G slabs stored pre-arranged as [qtl, p, lct, t, c] so gathered tiles
    # slice into contiguous DoubleRow stationaries
    gs0_d = nc.dram_tensor("gs0", [SLAB * HID], dt.float8e4)
    gf0_d = nc.dram_tensor("gf0", [N * HID], dt.float8e4, addr_space="Shared")
    gs12_d = nc.dram_tensor("gs12", [2 * SLAB * HID], dt.float8e4)
    gf12_d = nc.dram_tensor("gf12", [2 * N * HID], dt.float8e4,
                            addr_space="Shared")
    g2_slab = nc.dram_tensor("g2_slab", [SLAB, PAD2], dt.float8e4)
    g2_full = nc.dram_tensor("g2_full", [N, PAD2], dt.float8e4, addr_space="Shared")

    rg = [list(range(NCORES))]

    with tile.TileContext(nc) as tc:
      with (
          tc.tile_pool(name="adjt", bufs=NQT) as p_adjt,
          tc.tile_pool(name="xw", bufs=1) as p_xw,
          tc.tile_pool(name="small", bufs=1) as p_sm,
          tc.tile_pool(name="xct", bufs=1) as p_xct,
      ):
        # ---------------- x + u loads, s2 chain, s2 gather ------------------
        xsb = []
        xT_t = xT_d.rearrange("(ft p) i -> ft p i", p=128)
        for ft in range(NFT):
            t = p_xw.tile([128, SLAB], dt.bfloat16, tag="x", name="x", bufs=NFT)
            nc.sync.dma_start(t[:], xT_t[ft])
            xsb.append(t)
        u = p_sm.tile([128, NFT, 8], dt.bfloat16, tag="u", name="u")
        nc.scalar.dma_start(u[:], U_d.rearrange("(ft p) c -> p ft c", p=128))
        eye = p_sm.tile([128, 128], dt.float32, tag="eye", name="eye")
        nc.scalar.dma_start(eye[:], EYE_d[:])
        ones1 = p_sm.tile([1, 128], dt.float32, tag="ones1", name="ones1")
        nc.vector.memset(ones1[:], 1.0)

        s2loc = p_sm.tile([128, NIT, 4], dt.float32, tag="s2loc", name="s2loc")
        with tc.tile_pool(name="psS", bufs=2, space="PSUM") as ps_s:
            for it in range(NIT):
                ps = ps_s.tile([128, 8], dt.float32, tag="psS", name="psS")
                for ft in range(NFT):
                    nc.tensor.matmul(ps[:], xsb[ft][:, it * 128:(it + 1) * 128],
                                     u[:, ft, :],
                                     start=(ft == 0), stop=(ft == NFT - 1))
                nc.vector.tensor_copy(s2loc[:, it, 0:4], ps[:, 0:4])
                nc.sync.dma_start(s2s_d[it * 128:(it + 1) * 128, :],
                                  s2loc[:, it, :])
        nc.gpsimd.collective_compute(
            "AllGather", ALU.bypass, replica_groups=rg,
            ins=[s2s_d[:]], outs=[s2f_d[:]])

        # ---------------- bulk loads (overlap the s2 gather) ----------------
        wsb = [[None] * NFT for _ in range(NH)]
        W_t = W_d.rearrange("h (ft p) o -> h ft p o", p=128)
        for h in range(NH):
            for ft in range(NFT):
                t = p_xw.tile([128, HID], dt.bfloat16, tag="w", name="w",
                              bufs=NH * NFT)
                nc.scalar.dma_start(t[:], W_t[h, ft])
                wsb[h][ft] = t
        adjt = []
        adjT_t = adjT_d.rearrange("(qt t p) i -> qt p t i", t=2, p=128)
        for q in range(NQT):
            t = p_adjt.tile([128, 2, SLAB], dt.float8e4, tag="adjt", name="adjt")
            nc.scalar.dma_start(t[:], adjT_t[q])
            adjt.append(t)
        wo = []
        Wo_t = Wo_d.rearrange("(op t p) c -> op t p c", t=2, p=128)
        for op_ in range(NOT // 2):
            t = p_sm.tile([128, 2, G2C], dt.float8e4, tag="wo", name="wo",
                          bufs=NOT // 2)
            for tt in range(2):
                nc.scalar.dma_start(t[:, tt, :], Wo_t[op_, tt])
            wo.append(t)

        # ---------------- derive w from gathered s2 -------------------------
        # fast contiguous view for the global max (row mapping irrelevant)
        sfc = p_sm.tile([128, 32, 4], dt.float32, tag="sfc", name="sfc")
        nc.sync.dma_start(sfc[:], s2f_d.rearrange("(p r) c -> p r c", p=128))
        # strided j-partition view only feeds the (off-critical-path) W3
        sf = p_sm.tile([128, NJT, 4], dt.float32, tag="sf", name="sf")
        nc.scalar.dma_start(sf[:], s2f_d.rearrange("(jt p) c -> p jt c", p=128))
        negC3 = p_sm.tile([128, 4], dt.float32, tag="negC3", name="negC3")
        m1cat = p_sm.tile([128, 4], dt.float32, tag="m1cat", name="m1cat")
        nc.vector.memset(m1cat[:], -1e30)
        for h in range(NH):
            nc.vector.tensor_reduce(m1cat[:, h:h + 1], sfc[:, :, h],
                                    axis=mybir.AxisListType.X, op=ALU.max)
        with tc.tile_pool(name="psB1", bufs=1, space="PSUM") as ps_b:
            mT = ps_b.tile([128, SLAB], dt.float32, tag="psB", name="psB")
            nc.tensor.transpose(mT[0:4, 0:128], m1cat[:], eye[:])
            mcol = p_sm.tile([4, 1], dt.float32, tag="mcol", name="mcol")
            nc.vector.tensor_reduce(mcol[:], mT[0:4, 0:128],
                                    axis=mybir.AxisListType.X, op=ALU.max)
            mrowp = ps_b.tile([128, SLAB], dt.float32, tag="psB", name="psB")
            nc.tensor.matmul(mrowp[0:1, 0:4], mcol[:], eye[0:4, 0:4],
                             start=True, stop=True)
            negrow = p_sm.tile([1, 4], dt.float32, tag="negrow", name="negrow")
            nc.vector.tensor_scalar(negrow[:], mrowp[0:1, 0:4], -1.0, LOGC,
                                    ALU.mult, ALU.add)
            nbc = ps_b.tile([128, SLAB], dt.float32, tag="psB", name="psB")
            nc.tensor.matmul(nbc[0:128, 0:4], ones1[:], negrow[:],
                             start=True, stop=True)
            nc.vector.tensor_copy(negC3[:], nbc[0:128, 0:4])
        # fp8 denominator weights W3[j, h] = 16*exp(s2_j - C_h) for all j
        w3f = p_sm.tile([128, NJT, 16], dt.float32, tag="w3f", name="w3f")
        nc.vector.memset(w3f[:], 0.0)
        for h in range(NH):
            nc.scalar.activation(w3f[:, :, h], sf[:, :, h], AF.Exp,
                                 bias=negC3[:, h:h + 1])
        w3q = p_sm.tile([128, NJT, 16], dt.float8e4, tag="w3q", name="w3q")
        nc.vector.tensor_copy(w3q[:], w3f[:])
        # slab weights for scaling h into G
        w_sb = []
        for h in range(NH):
            w = p_sm.tile([128, NIT], dt.float32, tag="wexp", name="wexp",
                          bufs=NH)
            nc.scalar.activation(w[:], s2loc[:, :, h], AF.Exp,
                                 bias=negC3[:, h:h + 1])
            w_sb.append(w)

        # ---------------- per head: h = x@W, G = fp8(w*h), gather ----------
        gs0_v = gs0_d.rearrange("(i c) -> i c", c=HID)
        gs12_v = gs12_d.rearrange("(h i c) -> h i c", h=2, c=HID)
        with tc.tile_pool(name="psA", bufs=4, space="PSUM") as ps_a:
            for h in range(NH):
                for it in range(NIT):
                    ps = ps_a.tile([128, HID], dt.float32, tag="psA", name="psA")
                    for ft in range(NFT):
                        xh = xsb[ft][:, it * 128:(it + 1) * 128]
                        nc.tensor.matmul(ps[:, 0:512], xh, wsb[h][ft][:, 0:512],
                                         start=(ft == 0), stop=(ft == NFT - 1))
                        nc.tensor.matmul(ps[:, 512:HID], xh,
                                         wsb[h][ft][:, 512:HID],
                                         start=(ft == 0), stop=(ft == NFT - 1))
                    gq = p_sm.tile([128, HID], dt.float8e4, tag="gq",
                                   name="gq", bufs=4)
                    if it % 2 == 0:
                        nc.vector.tensor_scalar_mul(gq[:], ps[:],
                                                    w_sb[h][:, it:it + 1])
                    else:
                        nc.scalar.activation(gq[:], ps[:], AF.Copy,
                                             scale=w_sb[h][:, it:it + 1])
                    rows = slice(it * 128, (it + 1) * 128)
                    dst = (gs0_v[rows] if h == 0
                           else gs12_v[h - 1, rows])
                    nc.sync.dma_start(dst, gq[:])
                if h == 0:
                    nc.gpsimd.collective_compute(
                        "AllGather", ALU.bypass, replica_groups=rg,
                        ins=[gs0_d[:]], outs=[gf0_d[:]])
                elif h == 2:
                    nc.gpsimd.collective_compute(
                        "AllGather", ALU.bypass, replica_groups=rg,
                        ins=[gs12_d[:]], outs=[gf12_d[:]])

        # ------------- denominators: psd = W3^T @ A^T, reciprocal -----------
        rbc = []
        with tc.tile_pool(name="psDen", bufs=1, space="PSUM") as ps_b:
            psd = ps_b.tile([128, SLAB], dt.float32, tag="psB", name="psB")
            for q in range(NQT):
                nc.tensor.matmul(psd[0:16, :], w3q[:, 2 * q:2 * q + 2, :],
                                 adjt[q][:], start=(q == 0),
                                 stop=(q == NQT - 1), perf_mode=DR)
            recip3 = p_sm.tile([NH, SLAB], dt.float32, tag="recip3",
                               name="recip3")
            nc.vector.reciprocal(recip3[:], psd[0:NH, :])
            for h in range(NH):
                rrow = p_sm.tile([1, SLAB], dt.float32, tag="rrow",
                                 name="rrow", bufs=NH)
                nc.scalar.dma_start(rrow[:], recip3[h:h + 1, :])
                rbp = ps_b.tile([128, SLAB], dt.float32, tag="psB", name="psB")
                nc.tensor.matmul(rbp[:], ones1[:], rrow[:],
                                 start=True, stop=True)
                rb = p_sm.tile([128, SLAB], dt.float32, tag="rbc",
                               name="rbc", bufs=NH)
                nc.vector.tensor_copy(rb[:], rbp[:])
                rbc.append(rb)

        # ---- L1 adjacency matmuls + elu epilogue + incremental xcat@Wo -----
        xc = []
        with (
            tc.tile_pool(name="gst", bufs=48) as p_gst,
            tc.tile_pool(name="etmp", bufs=1) as p_et,
            tc.tile_pool(name="l2a", bufs=1) as p_l2a,
            tc.tile_pool(name="psB2", bufs=1, space="PSUM") as tail_psb,
        ):
          with (
            tc.tile_pool(name="ps1", bufs=3, space="PSUM") as ps_1,
            tc.tile_pool(name="psh2", bufs=4, space="PSUM") as ps_h2,
          ):
            ps2l = [ps_h2.tile([128, G2C], dt.float32, tag="psh2",
                               name="psh2") for _ in range(NIT)]
            gf0_v = gf0_d.rearrange("(q t p c) -> q p t c",
                                    t=2, p=128, c=HID)
            gf12_v = gf12_d.rearrange("(cc h q t p c) -> h cc q p t c",
                                      cc=NCORES, h=2, q=2, t=2, p=128)
            for h in range(NH):
                gt = []
                for q in range(NQT):
                    g = p_gst.tile([128, 2, HID], dt.float8e4,
                                   tag="gst", name="gst")
                    srcv = (gf0_v[q] if h == 0
                            else gf12_v[h - 1, q // 2, q % 2])
                    nc.sync.dma_start(g[:], srcv)
                    gt.append(g)
                for lct in range(NCT):
                    ps = ps_1.tile([128, SLAB], dt.float32, tag="ps1",
                                   name="ps1")
                    for q in range(NQT):
                        nc.tensor.matmul(
                            ps[:], gt[q][:, :, lct * 128:(lct + 1) * 128],
                            adjt[q][:], start=(q == 0), stop=(q == NQT - 1),
                            perf_mode=DR)
                    # xcatT tile = elu(numT / den) in bf16
                    z = p_et.tile([128, SLAB], dt.float32, tag="z",
                                  name="z", bufs=2)
                    nc.vector.tensor_tensor(z[:], ps[:], rbc[h][:], ALU.mult)
                    e = p_et.tile([128, SLAB], dt.float32, tag="e",
                                  name="e", bufs=2)
                    nc.scalar.activation(e[:], z[:], AF.Exp)
                    nc.vector.tensor_scalar(e[:], e[:], 1.0, -1.0,
                                            ALU.min, ALU.add)
                    ot = h * NCT + lct
                    if ot % 2 == 0:
                        xc.append(p_xct.tile([128, 2, SLAB], dt.float8e4,
                                             tag="xcp", name="xcp",
                                             bufs=NOT // 2))
                    th = xc[ot // 2][:, ot % 2, :]
                    nc.vector.scalar_tensor_tensor(th, z[:], 0.0, e[:],
                                                   ALU.max, ALU.add)
                # fold this head's xcat pairs into layer 2 immediately
                for op_ in range(h * NCT // 2, (h + 1) * NCT // 2):
                    for it in range(NIT):
                        nc.tensor.matmul(
                            ps2l[it][:],
                            xc[op_][:, :, it * 128:(it + 1) * 128],
                            wo[op_][:],
                            start=(op_ == 0), stop=(op_ == NOT // 2 - 1),
                            perf_mode=DR)

            # ---------------- layer 2 epilogue + masked softmax -------------
            s2p = p_l2a.tile([128, NIT], dt.float32, tag="s2p", name="s2p")
            h2_sb = []
            for it in range(NIT):
                h2 = p_l2a.tile([128, NCLS], dt.float32, tag="h2", name="h2",
                                bufs=NIT)
                nc.vector.tensor_copy(h2[:], ps2l[it][:, 0:NCLS])
                h2_sb.append(h2)
                nc.vector.tensor_copy(s2p[:, it:it + 1], ps2l[it][:, NCLS:G2C])
          # ps1/psh2 released; the tail below reuses those banks
          if True:
            # local max -> tiny parallel gather of the 8 per-core maxes
            sm1 = p_l2a.tile([128, 1], dt.float32, tag="sm1", name="sm1")
            nc.vector.tensor_reduce(sm1[:], s2p[:],
                                    axis=mybir.AxisListType.X, op=ALU.max)
            ps_b = tail_psb
            smT = ps_b.tile([128, SLAB], dt.float32, tag="psB", name="psB")
            nc.tensor.transpose(smT[0:1, 0:128], sm1[:], eye[:])
            c2loc = p_l2a.tile([1, 1], dt.float32, tag="c2loc", name="c2loc")
            nc.vector.tensor_reduce(c2loc[:], smT[0:1, 0:128],
                                    axis=mybir.AxisListType.X, op=ALU.max)
            negrow2 = p_l2a.tile([1, 1], dt.float32, tag="negrow2",
                                 name="negrow2")
            nc.vector.tensor_scalar(negrow2[:], c2loc[:], -1.0, LOGC,
                                    ALU.mult, ALU.add)
            nbc2 = ps_b.tile([128, SLAB], dt.float32, tag="psB", name="psB")
            nc.tensor.matmul(nbc2[0:128, 0:1], ones1[:], negrow2[:],
                             start=True, stop=True)
            negC2 = p_l2a.tile([128, 1], dt.float32, tag="negC2", name="negC2")
            nc.vector.tensor_copy(negC2[:], nbc2[0:128, 0:1])
            w2all = p_l2a.tile([128, NIT], dt.float32, tag="w2all", name="w2all")
            nc.scalar.activation(w2all[:], s2p[:], AF.Exp, bias=negC2[:])
            for it in range(NIT):
                rows = slice(it * 128, (it + 1) * 128)
                g2q = p_l2a.tile([128, PAD2], dt.float8e4, tag="g2q",
                                 name="g2q", bufs=2)
                nc.vector.tensor_scalar_mul(g2q[:, 0:NCLS], h2_sb[it][:],
                                            w2all[:, it:it + 1])
                nc.vector.tensor_copy(g2q[:, NCLS:G2C], w2all[:, it:it + 1])
                nc.vector.memset(g2q[:, G2C:PAD2], 0.0)
                if it == 0:
                    # smuggle the local softmax max in the pad bytes of row 0
                    nc.vector.tensor_copy(
                        g2q[0:1, 260:264],
                        c2loc[:].bitcast(dt.float8e4))
                nc.sync.dma_start(g2_slab[rows, :], g2q[:])
            nc.gpsimd.collective_compute(
                "AllGather", ALU.bypass, replica_groups=rg,
                ins=[g2_slab[:]], outs=[g2_full[:]])
            # rescale factors exp(C_local - C_global) per source slab
            cmq = p_l2a.tile([1, NCORES, 4], dt.float8e4, tag="cmq", name="cmq")
            nc.sync.dma_start(
                cmq[:],
                g2_full.rearrange("(cc r) b -> cc r b", r=SLAB)[:, 0,
                                                               260:264]
                .rearrange("cc b -> () cc b"))
            cm = cmq[:].bitcast(dt.float32).rearrange("o cc b -> o (cc b)")
            negCg = p_l2a.tile([1, 1], dt.float32, tag="negCg", name="negCg")
            nc.vector.tensor_reduce(negCg[:], cm[:],
                                    axis=mybir.AxisListType.X,
                                    op=ALU.max, negate=True)
            fr = p_l2a.tile([1, NCORES], dt.float32, tag="fr", name="fr")
            nc.scalar.activation(fr[:], cm[:], AF.Exp, bias=negCg[:])
            fbp = ps_b.tile([128, SLAB], dt.float32, tag="psB", name="psB")
            nc.tensor.matmul(fbp[0:128, 0:NCORES], ones1[:], fr[:],
                             start=True, stop=True)
            fbc = p_l2a.tile([128, NCORES], dt.float32, tag="fbc", name="fbc")
            nc.vector.tensor_copy(fbc[:], fbp[0:128, 0:NCORES])

            # L2 adjacency matmul + final epilogue
            with (
                tc.tile_pool(name="g2t", bufs=NQT) as p_g2t,
                tc.tile_pool(name="fin", bufs=1) as p_f,
                tc.tile_pool(name="ps2", bufs=4, space="PSUM") as ps_2,
            ):
                g2v = g2_full.rearrange("(qt t p) c -> qt p t c", t=2, p=128)
                g2t = []
                for q in range(NQT):
                    t = p_g2t.tile([128, 2, PAD2], dt.float8e4, tag="g2t",
                                   name="g2t")
                    nc.sync.dma_start(t[:], g2v[q])
                    g2t.append(t)
                for q in range(NQT):
                    for tt in range(2):
                        c = (2 * q + tt) // 4
                        nc.vector.tensor_scalar_mul(g2t[q][:, tt, :],
                                                    g2t[q][:, tt, :],
                                                    fbc[:, c:c + 1])
                ps2 = [ps_2.tile([128, PAD2], dt.float32, tag="ps2", name="ps2")
                       for _ in range(NIT)]
                for it in range(NIT):
                    for q in range(NQT):
                        nc.tensor.matmul(
                            ps2[it][:],
                            adjt[q][:, :, it * 128:(it + 1) * 128],
                            g2t[q][:], start=(q == 0), stop=(q == NQT - 1),
                            perf_mode=DR)
                r2s, zs, es, os, negms, ts, ssums, lgs = ({} for _ in range(8))
                for it in range(NIT):
                    r2s[it] = p_f.tile([128, 1], dt.float32, tag="r2",
                                       name="r2", bufs=NIT)
                    nc.vector.reciprocal(r2s[it][:], ps2[it][:, NCLS:G2C])
                for it in range(NIT):
                    # z and exp(z) both via scalar-engine scale (z = num/den)
                    zs[it] = p_f.tile([128, NCLS], dt.float32, tag="z2",
                                      name="z2", bufs=NIT)
                    nc.scalar.activation(zs[it][:], ps2[it][:, 0:NCLS],
                                         AF.Copy, scale=r2s[it][:])
                for it in range(NIT):
                    es[it] = p_f.tile([128, NCLS], dt.float32, tag="e2",
                                      name="e2", bufs=NIT)
                    nc.scalar.activation(es[it][:], ps2[it][:, 0:NCLS],
                                         AF.Exp, scale=r2s[it][:])
                for it in range(NIT):
                    nc.vector.tensor_scalar(es[it][:], es[it][:], 1.0, -1.0,
                                            ALU.min, ALU.add)
                for it in range(NIT):
                    os[it] = p_f.tile([128, NCLS], dt.float32, tag="o2",
                                      name="o2", bufs=NIT)
                    nc.vector.scalar_tensor_tensor(os[it][:], zs[it][:], 0.0,
                                                   es[it][:], ALU.max, ALU.add)
                # elu outputs are small enough that exp() is safe without
                # the usual max-subtraction
                for it in range(NIT):
                    ts[it] = p_f.tile([128, NCLS], dt.float32, tag="texp",
                                      name="texp", bufs=NIT)
                    nc.scalar.activation(ts[it][:], os[it][:], AF.Exp)
                for it in range(NIT):
                    ssums[it] = p_f.tile([128, 1], dt.float32, tag="ssum",
                                         name="ssum", bufs=NIT)
                    nc.vector.tensor_reduce(ssums[it][:], ts[it][:],
                                            axis=mybir.AxisListType.X,
                                            op=ALU.add)
                for it in range(NIT):
                    lgs[it] = p_f.tile([128, 1], dt.float32, tag="lg",
                                       name="lg", bufs=NIT)
                    nc.scalar.activation(lgs[it][:], ssums[it][:], AF.Ln)
                for it in range(NIT):
                    fin = p_f.tile([128, NCLS], dt.float32, tag="fin",
                                   name="fin", bufs=2)
                    nc.vector.tensor_scalar(fin[:], os[it][:], lgs[it][:], 0.0,
                                            ALU.subtract, ALU.bypass)
                    nc.sync.dma_start(out_d[it * 128:(it + 1) * 128, :], fin[:])

    nc.finalize()
    return nc


_CACHE = {}


def prepare_inputs(x, adj, W_heads, a_heads, W_out, a_out):
    """Shard + lay out the full inputs for the 8 cores."""
    x2 = np.asarray(x, np.float32)[0]          # [N, F]
    adj2 = np.asarray(adj)[0]                  # [N, N] int32
    W3 = np.asarray(W_heads, np.float32).reshape(NH, F, HID)
    a3 = np.asarray(a_heads, np.float32)       # [NH, 2*HID, 1]
    Wo = np.asarray(W_out, np.float32).reshape(GH, NCLS)
    ao = np.asarray(a_out, np.float32)         # [2*NCLS, 1]

    # fold the edge-score projections into the weights:
    #   s2 = x @ (W @ a2),   s2' = xcat @ (Wo @ ao2)
    u = np.einsum("hfo,ho->hf", W3.astype(np.float64),
                  a3[:, HID:, 0].astype(np.float64)).astype(np.float32)  # [NH,F]
    U = np.zeros((F, 8), BF16)
    for h in range(NH):
        U[:, h] = u[h].astype(BF16)
    u2 = (Wo.astype(np.float64) @ ao[NCLS:, 0].astype(np.float64)).astype(np.float32)
    Wo_ext = np.concatenate([Wo, u2[:, None]], axis=1).astype(F8)  # [GH, 257]
    Wb = W3.astype(BF16)
    xT = np.ascontiguousarray(x2.T)            # [F, N]
    adjb = adj2.astype(F8)                     # exact 0/1

    in_maps = []
    for c in range(NCORES):
        sl = slice(c * SLAB, (c + 1) * SLAB)
        in_maps.append({
            "adjT": np.ascontiguousarray(adjb[sl, :].T),
            "xT": np.ascontiguousarray(xT[:, sl]).astype(BF16),
            "U": U,
            "W": Wb,
            "Wo": Wo_ext,
            "EYE": np.eye(128, dtype=np.float32),
        })
    return in_maps


def kernel(x, adj, W_heads, a_heads, W_out, a_out):
    if "nc" not in _CACHE:
        # touch the devices once so any residual bad state from a previous
        # process surfaces (and clears) before the real run
        try:
            import jax
            jax.block_until_ready(jax.numpy.zeros(8))
        except Exception:
            pass
        _CACHE["nc"] = build()
    nc = _CACHE["nc"]
    in_maps = prepare_inputs(x, adj, W_heads, a_heads, W_out, a_out)
    res = run_bass_kernel_spmd(nc, in_maps, list(range(NCORES)))
    out = np.concatenate([res.results[c]["out"] for c in range(NCORES)], axis=0)
    return out.reshape(1, N, NCLS)
